# revision 18
# baseline (speedup 1.0000x reference)
"""MoE MLP (E=32 experts, top-2, D=H=1024) on 8 Trainium2 NeuronCores.

Strategy (expert parallel, per sharding hint):
  * Host computes the (tiny) gate: softmax(x @ Wg), top-2, renormalized
    weights, and dispatches tokens per expert into capacity-padded blocks,
    transposed to [D, tokens] (features on SBUF partitions, tokens on the
    matmul moving/free dimension). This is the sharding/all-to-all step.
  * Experts are snake-assigned to (core, position) slots by descending
    token count, so position j has a tight per-position capacity
    caps[j] = max over cores of that position's count (rounded up).  All
    cores run the SAME program (SPMD) with per-position capacities,
    cutting capacity padding from 4*max_e to sum_j caps[j].
  * Each core computes GELU(x W1 + b1) W2 + b2 for its 4 experts' blocks.
  * Host combines with the top-2 gate weights (scatter-add).

Device kernel notes:
  * Weights are host-pre-tiled to [e, col_tile, partition, k_tile, 128] so
    each half-layer streams in as one fully-contiguous DMA chunk on the
    sync (HWDGE) queue, in exact consumption order.
  * First expert's x block goes on the scalar HWDGE queue (fast first
    byte) so real matmuls can start ~8us in; later x blocks prefetch on
    gpsimd.
  * Warmup: a short chain of small dummy matmuls keeps the PE busy from
    the end of the framework preamble until the first data lands, and
    starts the HAM clock-warm window early.  A dummy activation preloads
    the GELU table during the same window.
  * y is stored as bf16 (halves write traffic) in 2-dt-tile groups so the
    final store after the last matmul is small.
"""

import os
import sys
import numpy as np

for _p in ("/root/.axon_site/_ro/trn_rl_repo", "/opt/trn_rl_repo"):
    if _p not in sys.path and os.path.isdir(_p):
        sys.path.append(_p)

E, D, H = 32, 1024, 1024
TOP_K = 2
N_CORES = 8
EPC = E // N_CORES  # experts per core (= positions)
ND = D // 128       # d 128-tiles
NH = H // 128       # h 128-tiles

# weight dtype, activation dtype (must both be 16-bit or both 32-bit)
DT_W = os.environ.get("MOE_DT_W", "bfloat16")
DT_A = os.environ.get("MOE_DT_A", "bfloat16")
DT_Y = os.environ.get("MOE_DT_Y", "bfloat16")
N_WARMUP_MM = int(os.environ.get("MOE_WARMUP", "34"))
WU_N = int(os.environ.get("MOE_WU_N", "128"))
CAP_GRAN = int(os.environ.get("MOE_CAP_GRAN", "4"))

LAST_EXEC_TIME_NS = None

_NC_CACHE = {}


def _chunks(cap):
    """Split a capacity into <=512-wide chunks (usually just one)."""
    tch = -(-cap // 512)
    base = -(-cap // (tch * CAP_GRAN)) * CAP_GRAN
    out = []
    left = cap
    while left > 0:
        w = min(base, left)
        out.append(w)
        left -= w
    return out


def _build_nc(caps, dt_w_name, dt_a_name, dt_y_name):
    import concourse.bass as bass  # noqa: F401
    import concourse.tile as tile
    from concourse import bacc, mybir
    from contextlib import ExitStack

    f32 = mybir.dt.float32
    dt_w = getattr(mybir.dt, dt_w_name)
    dt_a = getattr(mybir.dt, dt_a_name)
    dt_y = getattr(mybir.dt, dt_y_name)
    CTOT = sum(caps)
    offs = [0]
    for c in caps:
        offs.append(offs[-1] + c)
    cap_max = max(caps)

    nc = bacc.Bacc(
        "TRN2",
        target_bir_lowering=False,
        debug=False,
        enable_asserts=False,
        num_devices=N_CORES,
    )
    xT = nc.dram_tensor("xT", [D, CTOT], dt_a, kind="ExternalInput").ap()
    # host-pre-tiled: w1[e, ht, p(=d_in), dt, hi], w2[e, dt, p(=h_in), ht, di]
    w1 = nc.dram_tensor("w1", [EPC, NH, 128, ND, 128], dt_w, kind="ExternalInput").ap()
    w2 = nc.dram_tensor("w2", [EPC, ND, 128, NH, 128], dt_w, kind="ExternalInput").ap()
    # host-pre-transposed biases: [p, e, col_tile]
    b1 = nc.dram_tensor("b1", [128, EPC, NH], f32, kind="ExternalInput").ap()
    b2 = nc.dram_tensor("b2", [128, EPC, ND], f32, kind="ExternalInput").ap()
    yT = nc.dram_tensor("yT", [D, CTOT], dt_y, kind="ExternalOutput").ap()

    HNH = NH // 2  # half-layer column split
    HND = ND // 2
    WB = 3 if mybir.dt.size(dt_w) == 4 else 4

    with tile.TileContext(nc) as tc, ExitStack() as ctx:
        wpool = ctx.enter_context(tc.tile_pool(name="w", bufs=4))
        xpool = ctx.enter_context(tc.tile_pool(name="x", bufs=EPC))
        hpool = ctx.enter_context(tc.tile_pool(name="h", bufs=2 * NH))
        ypool = ctx.enter_context(tc.tile_pool(name="y", bufs=EPC + 1))
        bpool = ctx.enter_context(tc.tile_pool(name="b", bufs=1))
        pp1 = ctx.enter_context(tc.tile_pool(name="ps1", bufs=3, space="PSUM"))
        pp2 = ctx.enter_context(tc.tile_pool(name="ps2", bufs=3, space="PSUM"))
        ppw = ctx.enter_context(tc.tile_pool(name="psw", bufs=1, space="PSUM"))

        gelu = mybir.ActivationFunctionType.Gelu

        # PE warm-up: small dummy matmuls with no DMA dependency keep the
        # PE busy from the end of the engine preamble until the first
        # weights/x land, and open the HAM busy window early.
        if N_WARMUP_MM:
            wu = bpool.tile([128, WU_N], mybir.dt.bfloat16, tag="wu")
            nc.gpsimd.memset(wu[:], 0.0)
            wups = ppw.tile([128, WU_N], f32, tag="psw")
            for i in range(N_WARMUP_MM):
                nc.tensor.matmul(wups[:], wu[:, :128], wu[:],
                                 start=(i == 0), stop=(i == N_WARMUP_MM - 1))

        b1_sb = b2_sb = None
        ysbs = []
        for e in range(EPC):
            cap = caps[e]
            off = offs[e]
            # ALL weight loads go on the single sync HWDGE ring in EXACT
            # consumption order (W1_e chunks, then W2_e chunks, then
            # x_{e+1}, W1_{e+1}, ...).  The ring is FIFO, so HBM bandwidth
            # always goes to the next-needed byte — no competing streams.
            # Exception: x0 rides the scalar ring, concurrently with W1_0's
            # first chunks — both are needed at compute start.
            xt = xpool.tile([128, ND * cap], dt_a, tag="xt")
            (nc.scalar if e == 0 else nc.sync).dma_start(
                out=xt[:].rearrange("p (dt t) -> p dt t", dt=ND),
                in_=xT[:, off:off + cap].rearrange("(dt p) t -> p dt t", p=128),
            )
            if e == 0 and N_WARMUP_MM:
                # preload the GELU table on the scalar engine while it
                # idles (after x0's trigger so it doesn't delay it)
                wuact = bpool.tile([128, 1], f32, tag="wuact")
                nc.scalar.activation(wuact[:], wu[:, :1], gelu)
            # W1 in column chunks: a small first chunk for the first expert
            # so compute starts ASAP and the rest pipelines.
            w1_chunks = [1, 2, 2, 3] if e == 0 else [NH // 2, NH // 2]
            w1h = []     # per h-tile: (tile, h-tile offset within tile)
            h0 = 0
            for csz in w1_chunks:
                wt = wpool.tile([128, csz * ND * 128], dt_w,
                                tag=f"w1s{csz}",
                                bufs=(2 if e == 0 else WB))
                nc.sync.dma_start(
                    out=wt[:].rearrange("p (ht dt hi) -> p ht dt hi", ht=csz, dt=ND),
                    in_=w1[e, h0:h0 + csz].rearrange("ht p dt hi -> p ht dt hi"),
                )
                for k in range(csz):
                    w1h.append((wt, k))
                h0 += csz
            if b1_sb is None:
                b1_sb = bpool.tile([128, EPC * NH], f32, tag="b1")
                b2_sb = bpool.tile([128, EPC * ND], f32, tag="b2")
                nc.gpsimd.dma_start(
                    out=b1_sb[:].rearrange("p (e ht) -> p e ht", e=EPC), in_=b1[:])
                nc.gpsimd.dma_start(
                    out=b2_sb[:].rearrange("p (e dt) -> p e dt", e=EPC), in_=b2[:])
            # W2: halves, but quarters for the LAST expert so the final
            # weight chunk (and thus the compute tail after the last DMA
            # byte) is as short as possible.
            w2_chunks = [2, 2, 2, 2] if e == EPC - 1 else [HND, HND]
            w2h = []     # per dt-tile: (tile, dt offset within tile)
            d0 = 0
            for csz in w2_chunks:
                wt = wpool.tile([128, csz * NH * 128], dt_w,
                                tag=f"w2s{csz}", bufs=WB)
                nc.sync.dma_start(
                    out=wt[:].rearrange("p (dt ht di) -> p dt ht di", dt=csz, ht=NH),
                    in_=w2[e, d0:d0 + csz].rearrange("dt p ht di -> p dt ht di"),
                )
                for k in range(csz):
                    w2h.append((wt, k))
                d0 += csz

            ch0 = 0
            for CW in _chunks(cap):
                hts = []
                for ht in range(NH):
                    wt, hk = w1h[ht]
                    hoff = hk * ND * 128
                    ps = pp1.tile([128, CW], f32, tag="ps1")
                    for dt_i in range(ND):
                        nc.tensor.matmul(
                            ps[:],
                            wt[:, hoff + dt_i * 128: hoff + (dt_i + 1) * 128],
                            xt[:, dt_i * cap + ch0: dt_i * cap + ch0 + CW],
                            start=(dt_i == 0),
                            stop=(dt_i == ND - 1),
                        )
                    hsb = hpool.tile([128, CW], dt_a, tag="ht")
                    nc.scalar.activation(
                        hsb[:], ps[:], gelu,
                        bias=b1_sb[:, e * NH + ht: e * NH + ht + 1],
                    )
                    hts.append(hsb)
                ysb = ypool.tile([128, ND * CW], dt_y, tag="yt")
                for dt_i in range(ND):
                    wt, dk = w2h[dt_i]
                    doff = dk * NH * 128
                    ps2 = pp2.tile([128, CW], f32, tag="ps2")
                    for ht in range(NH):
                        nc.tensor.matmul(
                            ps2[:],
                            wt[:, doff + ht * 128: doff + (ht + 1) * 128],
                            hts[ht][:],
                            start=(ht == 0),
                            stop=(ht == NH - 1),
                        )
                    nc.vector.tensor_scalar_add(
                        ysb[:, dt_i * CW:(dt_i + 1) * CW], ps2[:],
                        b2_sb[:, e * ND + dt_i: e * ND + dt_i + 1],
                    )
                ysbs.append((ysb, e, ch0, CW))
                ch0 += CW

        # y stores, all deferred to the END of the sync engine's queue.
        # The sync HWDGE ring is FIFO, so these transfers land strictly
        # after every weight byte — y writes never steal HBM bandwidth
        # from the weight stream, and instead overlap the final compute
        # and the kernel-exit barrier.  The very last piece (the final
        # expert's last 2 d-tiles) goes on the scalar ring so its small
        # transfer + completion receipt runs in parallel with the sync
        # ring's, right after the last matmul.
        for i, (ysb, e, ch0, CW) in enumerate(ysbs):
            col = offs[e] + ch0
            if i == len(ysbs) - 1:
                sp = ND - 2
                nc.sync.dma_start(
                    out=yT[:sp * 128, col: col + CW]
                    .rearrange("(dt p) t -> p dt t", p=128),
                    in_=ysb[:, :sp * CW].rearrange("p (dt t) -> p dt t", dt=sp),
                )
                nc.scalar.dma_start(
                    out=yT[sp * 128:, col: col + CW]
                    .rearrange("(dt p) t -> p dt t", p=128),
                    in_=ysb[:, sp * CW:].rearrange("p (dt t) -> p dt t", dt=ND - sp),
                )
            else:
                nc.sync.dma_start(
                    out=yT[:, col: col + CW]
                    .rearrange("(dt p) t -> p dt t", p=128),
                    in_=ysb[:].rearrange("p (dt t) -> p dt t", dt=ND),
                )
    nc.compile()
    return nc


def _get_nc(caps, dt_w, dt_a, dt_y):
    key = (caps, dt_w, dt_a, dt_y, N_WARMUP_MM, WU_N)
    if key not in _NC_CACHE:
        _NC_CACHE[key] = _build_nc(caps, dt_w, dt_a, dt_y)
    return _NC_CACHE[key]


def _np_dt(name):
    if name == "bfloat16":
        import ml_dtypes
        return np.dtype(ml_dtypes.bfloat16)
    return np.dtype(np.float32)


def _route(xf, Wg):
    """Replicates the reference gate exactly in f32 numpy."""
    logits = xf @ Wg                                     # [T, E]
    m = logits.max(-1, keepdims=True)
    ex = np.exp(logits - m)
    scores = ex / ex.sum(-1, keepdims=True)
    idx = np.argsort(-scores, axis=1, kind="stable")[:, :TOP_K]  # [T, k]
    tw = np.take_along_axis(scores, idx, 1)
    m2 = tw.max(-1, keepdims=True)
    e2 = np.exp(tw - m2)
    w = (e2 / e2.sum(-1, keepdims=True)).astype(np.float32)
    return idx.astype(np.int64), w


def kernel(x, Wg, W1, b1, W2, b2):
    global LAST_EXEC_TIME_NS
    from concourse import bass_utils

    dt_w, dt_a, dt_y = DT_W, DT_A, DT_Y
    orig_shape = x.shape
    x = np.asarray(x, dtype=np.float32)
    Wg = np.asarray(Wg, dtype=np.float32)
    W1 = np.asarray(W1, dtype=np.float32)
    b1 = np.asarray(b1, dtype=np.float32)
    W2 = np.asarray(W2, dtype=np.float32)
    b2 = np.asarray(b2, dtype=np.float32)
    xf = np.ascontiguousarray(x.reshape(-1, D))
    T = xf.shape[0]

    idx, w = _route(xf, Wg)

    # ---- balanced expert->(core, position) assignment (snake by count)
    flat_e = idx.reshape(-1)                 # [k*T]
    flat_t = np.repeat(np.arange(T), TOP_K)
    counts = np.bincount(flat_e, minlength=E)
    order_e = np.argsort(-counts, kind="stable")
    assign = np.empty((N_CORES, EPC), np.int64)   # assign[c, j] = expert id
    for j in range(EPC):
        blk = order_e[j * N_CORES:(j + 1) * N_CORES]
        assign[:, j] = blk if j % 2 == 0 else blk[::-1]
    caps = []
    for j in range(EPC):
        mx = int(counts[assign[:, j]].max())
        caps.append(max(16, -(-mx // CAP_GRAN) * CAP_GRAN))
    caps = tuple(caps)
    offs = np.zeros(EPC + 1, np.int64)
    offs[1:] = np.cumsum(caps)
    CTOT = int(offs[-1])
    core_of = np.empty(E, np.int64)
    pos_of = np.empty(E, np.int64)
    for c in range(N_CORES):
        for j in range(EPC):
            core_of[assign[c, j]] = c
            pos_of[assign[c, j]] = j

    # ---- dispatch: per-expert capacity-padded token blocks
    order = np.argsort(flat_e, kind="stable")
    starts = np.zeros(E + 1, np.int64)
    starts[1:] = np.cumsum(counts)
    se = flat_e[order]
    pos = np.arange(TOP_K * T) - starts[se]
    core = core_of[se]
    col = offs[pos_of[se]] + pos             # column in that core's xT
    tok = flat_t[order]

    gidx = np.zeros((N_CORES, CTOT), np.int64)
    for c in range(N_CORES):
        msel = core == c
        gidx[c, col[msel]] = tok[msel]

    np_w = _np_dt(dt_w)
    np_a = _np_dt(dt_a)
    xf_a = xf.astype(np_a, copy=False)
    # pre-tile weights: w1 -> [e, ht, p(d_in), dt, hi], w2 -> [e, dt, p(h_in), ht, di]
    W1t = W1.reshape(E, ND, 128, NH, 128).transpose(0, 3, 2, 1, 4).astype(np_w, copy=False)
    W2t = W2.reshape(E, NH, 128, ND, 128).transpose(0, 3, 2, 1, 4).astype(np_w, copy=False)
    # pre-transpose biases to [p, e, col_tile]
    b1t = b1.reshape(E, NH, 128).transpose(2, 0, 1)
    b2t = b2.reshape(E, ND, 128).transpose(2, 0, 1)

    in_maps = []
    for c in range(N_CORES):
        es = assign[c]
        in_maps.append({
            "xT": np.ascontiguousarray(xf_a[gidx[c]].T),
            "w1": np.ascontiguousarray(W1t[es]),
            "w2": np.ascontiguousarray(W2t[es]),
            "b1": np.ascontiguousarray(b1t[:, es]),
            "b2": np.ascontiguousarray(b2t[:, es]),
        })

    nc = _get_nc(caps, dt_w, dt_a, dt_y)
    trace = os.environ.get("MOE_TRACE", "0") == "1"
    res = bass_utils.run_bass_kernel_spmd(
        nc, in_maps, core_ids=list(range(N_CORES)), trace=trace,
    )
    LAST_EXEC_TIME_NS = res.exec_time_ns

    # ---- combine: gather each (token, k) contribution, weight, and sum
    Ystack = np.stack([np.asarray(res.results[c]["yT"], dtype=np.float32).T
                       for c in range(N_CORES)])
    contrib = Ystack[core, col]              # [k*T, D] (sorted order)
    inv = np.empty_like(order)
    inv[order] = np.arange(TOP_K * T)
    contrib = contrib[inv].reshape(T, TOP_K, D)
    y = (contrib * w[:, :, None]).sum(1).astype(np.float32)
    return y.reshape(orig_shape)


# revision 26
# speedup vs baseline: 1.0467x; 1.0467x over previous
"""MoE MLP (E=32 experts, top-2, D=H=1024) on 8 Trainium2 NeuronCores.

Strategy (expert parallel, per sharding hint):
  * Host computes the (tiny) gate: softmax(x @ Wg), top-2, renormalized
    weights, and dispatches tokens per expert into capacity-padded blocks,
    transposed to [D, tokens] (features on SBUF partitions, tokens on the
    matmul moving/free dimension). This is the sharding/all-to-all step.
  * Experts are snake-assigned to (core, position) slots by descending
    token count, so position j has a tight per-position capacity
    caps[j] = max over cores of that position's count (rounded up).  All
    cores run the SAME program (SPMD) with per-position capacities,
    cutting capacity padding from 4*max_e to sum_j caps[j].
  * Each core computes GELU(x W1 + b1) W2 + b2 for its 4 experts' blocks.
  * Host combines with the top-2 gate weights (scatter-add).

Device kernel notes:
  * Weights are host-pre-tiled to [e, col_tile, partition, k_tile, 128] so
    each half-layer streams in as one fully-contiguous DMA chunk on the
    sync (HWDGE) queue, in exact consumption order.
  * First expert's x block goes on the scalar HWDGE queue (fast first
    byte) so real matmuls can start ~8us in; later x blocks prefetch on
    gpsimd.
  * Warmup: a short chain of small dummy matmuls keeps the PE busy from
    the end of the framework preamble until the first data lands, and
    starts the HAM clock-warm window early.  A dummy activation preloads
    the GELU table during the same window.
  * y is stored as bf16 (halves write traffic) in 2-dt-tile groups so the
    final store after the last matmul is small.
"""

import os
import sys
import numpy as np

for _p in ("/root/.axon_site/_ro/trn_rl_repo", "/opt/trn_rl_repo"):
    if _p not in sys.path and os.path.isdir(_p):
        sys.path.append(_p)

E, D, H = 32, 1024, 1024
TOP_K = 2
N_CORES = 8
EPC = E // N_CORES  # experts per core (= positions)
ND = D // 128       # d 128-tiles
NH = H // 128       # h 128-tiles

# weight dtype, activation dtype (must both be 16-bit or both 32-bit)
DT_W = os.environ.get("MOE_DT_W", "bfloat16")
DT_A = os.environ.get("MOE_DT_A", "bfloat16")
DT_Y = os.environ.get("MOE_DT_Y", "bfloat16")
N_WARMUP_MM = int(os.environ.get("MOE_WARMUP", "48"))
WU_N = int(os.environ.get("MOE_WU_N", "128"))
CAP_GRAN = int(os.environ.get("MOE_CAP_GRAN", "4"))

LAST_EXEC_TIME_NS = None

_NC_CACHE = {}


def _chunks(cap):
    """Split a capacity into <=512-wide chunks (usually just one)."""
    tch = -(-cap // 512)
    base = -(-cap // (tch * CAP_GRAN)) * CAP_GRAN
    out = []
    left = cap
    while left > 0:
        w = min(base, left)
        out.append(w)
        left -= w
    return out


def _build_nc(caps, dt_w_name, dt_a_name, dt_y_name):
    import concourse.bass as bass  # noqa: F401
    import concourse.tile as tile
    from concourse import bacc, mybir
    from contextlib import ExitStack

    f32 = mybir.dt.float32
    dt_w = getattr(mybir.dt, dt_w_name)
    dt_a = getattr(mybir.dt, dt_a_name)
    dt_y = getattr(mybir.dt, dt_y_name)
    CTOT = sum(caps)
    offs = [0]
    for c in caps:
        offs.append(offs[-1] + c)
    cap_max = max(caps)

    nc = bacc.Bacc(
        "TRN2",
        target_bir_lowering=False,
        debug=False,
        enable_asserts=False,
        num_devices=N_CORES,
    )
    # All DRAM layouts are partition-major so every DMA writes/reads one
    # long contiguous run per SBUF partition (~4-8KB descriptors instead
    # of ~500B token-rows; small HBM descriptors measured ~5x slower).
    #   xT[p, s]  where s = ND*offs[e] + dt*cap_e + t
    #   w1[e, p, ht, dt, hi],  w2[e, p, dt, ht, di]
    #   yT[p, s]  same s-indexing as xT
    xT = nc.dram_tensor("xT", [128, ND * CTOT], dt_a, kind="ExternalInput").ap()
    w1 = nc.dram_tensor("w1", [EPC, 128, NH, ND, 128], dt_w, kind="ExternalInput").ap()
    w2 = nc.dram_tensor("w2", [EPC, 128, ND, NH, 128], dt_w, kind="ExternalInput").ap()
    # host-pre-transposed biases: [p, e, col_tile]
    b1 = nc.dram_tensor("b1", [128, EPC, NH], f32, kind="ExternalInput").ap()
    b2 = nc.dram_tensor("b2", [128, EPC, ND], f32, kind="ExternalInput").ap()
    yT = nc.dram_tensor("yT", [128, ND * CTOT], dt_y, kind="ExternalOutput").ap()

    HNH = NH // 2  # half-layer column split
    HND = ND // 2
    WB = 3 if mybir.dt.size(dt_w) == 4 else 4

    with tile.TileContext(nc) as tc, ExitStack() as ctx:
        wpool = ctx.enter_context(tc.tile_pool(name="w", bufs=4))
        xpool = ctx.enter_context(tc.tile_pool(name="x", bufs=EPC))
        hpool = ctx.enter_context(tc.tile_pool(name="h", bufs=2 * NH))
        ypool = ctx.enter_context(tc.tile_pool(name="y", bufs=EPC + 1))
        bpool = ctx.enter_context(tc.tile_pool(name="b", bufs=1))
        pp1 = ctx.enter_context(tc.tile_pool(name="ps1", bufs=3, space="PSUM"))
        pp2 = ctx.enter_context(tc.tile_pool(name="ps2", bufs=3, space="PSUM"))
        ppw = ctx.enter_context(tc.tile_pool(name="psw", bufs=1, space="PSUM"))

        gelu = mybir.ActivationFunctionType.Gelu

        # PE warm-up: small dummy matmuls with no DMA dependency keep the
        # PE busy from the end of the engine preamble until the first
        # weights/x land, and open the HAM busy window early.
        if N_WARMUP_MM:
            wu = bpool.tile([128, WU_N], mybir.dt.bfloat16, tag="wu")
            nc.gpsimd.memset(wu[:], 0.0)
            wups = ppw.tile([128, WU_N], f32, tag="psw")
            for i in range(N_WARMUP_MM):
                nc.tensor.matmul(wups[:], wu[:, :128], wu[:],
                                 start=(i == 0), stop=(i == N_WARMUP_MM - 1))

        b1_sb = b2_sb = None
        ysbs = []
        for e in range(EPC):
            cap = caps[e]
            off = offs[e]
            # ALL weight loads go on the single sync HWDGE ring in EXACT
            # consumption order (W1_e chunks, then W2_e chunks, then
            # x_{e+1}, W1_{e+1}, ...).  The ring is FIFO, so HBM bandwidth
            # always goes to the next-needed byte — no competing streams.
            # Exception: x0 rides the scalar ring, concurrently with W1_0's
            # first chunks — both are needed at compute start.
            xt = xpool.tile([128, ND * cap], dt_a, tag="xt")
            (nc.scalar if e == 0 else nc.sync).dma_start(
                out=xt[:],
                in_=xT[:, ND * off: ND * (off + cap)],
            )
            if e == 0 and N_WARMUP_MM:
                # preload the GELU table on the scalar engine while it
                # idles (after x0's trigger so it doesn't delay it)
                wuact = bpool.tile([128, 1], f32, tag="wuact")
                nc.scalar.activation(wuact[:], wu[:, :1], gelu)
            # W1 in column chunks: a small first chunk for the first expert
            # so compute starts ASAP and the rest pipelines.
            w1_chunks = [1, 2, 2, 3] if e == 0 else [NH // 2, NH // 2]
            w1h = []     # per h-tile: (tile, h-tile offset within tile)
            h0 = 0
            for csz in w1_chunks:
                wt = wpool.tile([128, csz * ND * 128], dt_w,
                                tag=f"w1s{csz}",
                                bufs=(2 if e == 0 else WB))
                nc.sync.dma_start(
                    out=wt[:],
                    in_=w1[e, :, h0:h0 + csz].rearrange("p ht dt hi -> p (ht dt hi)"),
                )
                for k in range(csz):
                    w1h.append((wt, k))
                h0 += csz
            if b1_sb is None:
                b1_sb = bpool.tile([128, EPC * NH], f32, tag="b1")
                b2_sb = bpool.tile([128, EPC * ND], f32, tag="b2")
                nc.gpsimd.dma_start(
                    out=b1_sb[:].rearrange("p (e ht) -> p e ht", e=EPC), in_=b1[:])
                nc.gpsimd.dma_start(
                    out=b2_sb[:].rearrange("p (e dt) -> p e dt", e=EPC), in_=b2[:])
            # W2: halves, but quarters for the LAST expert so the final
            # weight chunk (and thus the compute tail after the last DMA
            # byte) is as short as possible.
            w2_chunks = [2, 2, 2, 2] if e == EPC - 1 else [HND, HND]
            w2h = []     # per dt-tile: (tile, dt offset within tile)
            d0 = 0
            for csz in w2_chunks:
                wt = wpool.tile([128, csz * NH * 128], dt_w,
                                tag=f"w2s{csz}", bufs=WB)
                nc.sync.dma_start(
                    out=wt[:],
                    in_=w2[e, :, d0:d0 + csz].rearrange("p dt ht di -> p (dt ht di)"),
                )
                for k in range(csz):
                    w2h.append((wt, k))
                d0 += csz

            ch0 = 0
            for CW in _chunks(cap):
                hts = []
                for ht in range(NH):
                    wt, hk = w1h[ht]
                    hoff = hk * ND * 128
                    ps = pp1.tile([128, CW], f32, tag="ps1")
                    for dt_i in range(ND):
                        nc.tensor.matmul(
                            ps[:],
                            wt[:, hoff + dt_i * 128: hoff + (dt_i + 1) * 128],
                            xt[:, dt_i * cap + ch0: dt_i * cap + ch0 + CW],
                            start=(dt_i == 0),
                            stop=(dt_i == ND - 1),
                        )
                    hsb = hpool.tile([128, CW], dt_a, tag="ht")
                    nc.scalar.activation(
                        hsb[:], ps[:], gelu,
                        bias=b1_sb[:, e * NH + ht: e * NH + ht + 1],
                    )
                    hts.append(hsb)
                ysb = ypool.tile([128, ND * CW], dt_y, tag="yt")
                for dt_i in range(ND):
                    wt, dk = w2h[dt_i]
                    doff = dk * NH * 128
                    ps2 = pp2.tile([128, CW], f32, tag="ps2")
                    for ht in range(NH):
                        nc.tensor.matmul(
                            ps2[:],
                            wt[:, doff + ht * 128: doff + (ht + 1) * 128],
                            hts[ht][:],
                            start=(ht == 0),
                            stop=(ht == NH - 1),
                        )
                    nc.vector.tensor_scalar_add(
                        ysb[:, dt_i * CW:(dt_i + 1) * CW], ps2[:],
                        b2_sb[:, e * ND + dt_i: e * ND + dt_i + 1],
                    )
                ysbs.append((ysb, e, ch0, CW))
                ch0 += CW

        # y stores, all deferred to the END of the sync engine's queue.
        # The sync HWDGE ring is FIFO, so these transfers land strictly
        # after every weight byte — y writes never steal HBM bandwidth
        # from the weight stream, and instead overlap the final compute
        # and the kernel-exit barrier.  The very last piece (the final
        # expert's last 2 d-tiles) goes on the scalar ring so its small
        # transfer + completion receipt runs in parallel with the sync
        # ring's, right after the last matmul.
        for i, (ysb, e, ch0, CW) in enumerate(ysbs):
            cap = caps[e]
            base = ND * offs[e]
            if CW != cap:
                # multi-chunk expert (cap > 512): per-d-tile strided stores
                for dt_i in range(ND):
                    nc.sync.dma_start(
                        out=yT[:, base + dt_i * cap + ch0:
                               base + dt_i * cap + ch0 + CW],
                        in_=ysb[:, dt_i * CW:(dt_i + 1) * CW],
                    )
            elif i == len(ysbs) - 1:
                sp = ND - 2
                nc.sync.dma_start(
                    out=yT[:, base: base + sp * cap],
                    in_=ysb[:, :sp * CW],
                )
                nc.scalar.dma_start(
                    out=yT[:, base + sp * cap: base + ND * cap],
                    in_=ysb[:, sp * CW:],
                )
            else:
                nc.sync.dma_start(
                    out=yT[:, base: base + ND * cap],
                    in_=ysb[:],
                )
    nc.compile()
    return nc


def _get_nc(caps, dt_w, dt_a, dt_y):
    key = (caps, dt_w, dt_a, dt_y, N_WARMUP_MM, WU_N)
    if key not in _NC_CACHE:
        _NC_CACHE[key] = _build_nc(caps, dt_w, dt_a, dt_y)
    return _NC_CACHE[key]


def _np_dt(name):
    if name == "bfloat16":
        import ml_dtypes
        return np.dtype(ml_dtypes.bfloat16)
    return np.dtype(np.float32)


def _route(xf, Wg):
    """Replicates the reference gate exactly in f32 numpy."""
    logits = xf @ Wg                                     # [T, E]
    m = logits.max(-1, keepdims=True)
    ex = np.exp(logits - m)
    scores = ex / ex.sum(-1, keepdims=True)
    idx = np.argsort(-scores, axis=1, kind="stable")[:, :TOP_K]  # [T, k]
    tw = np.take_along_axis(scores, idx, 1)
    m2 = tw.max(-1, keepdims=True)
    e2 = np.exp(tw - m2)
    w = (e2 / e2.sum(-1, keepdims=True)).astype(np.float32)
    return idx.astype(np.int64), w


def kernel(x, Wg, W1, b1, W2, b2):
    global LAST_EXEC_TIME_NS
    from concourse import bass_utils

    dt_w, dt_a, dt_y = DT_W, DT_A, DT_Y
    orig_shape = x.shape
    x = np.asarray(x, dtype=np.float32)
    Wg = np.asarray(Wg, dtype=np.float32)
    W1 = np.asarray(W1, dtype=np.float32)
    b1 = np.asarray(b1, dtype=np.float32)
    W2 = np.asarray(W2, dtype=np.float32)
    b2 = np.asarray(b2, dtype=np.float32)
    xf = np.ascontiguousarray(x.reshape(-1, D))
    T = xf.shape[0]

    idx, w = _route(xf, Wg)

    # ---- balanced expert->(core, position) assignment (snake by count)
    flat_e = idx.reshape(-1)                 # [k*T]
    flat_t = np.repeat(np.arange(T), TOP_K)
    counts = np.bincount(flat_e, minlength=E)
    order_e = np.argsort(-counts, kind="stable")
    assign = np.empty((N_CORES, EPC), np.int64)   # assign[c, j] = expert id
    for j in range(EPC):
        blk = order_e[j * N_CORES:(j + 1) * N_CORES]
        assign[:, j] = blk if j % 2 == 0 else blk[::-1]
    caps = []
    for j in range(EPC):
        mx = int(counts[assign[:, j]].max())
        caps.append(max(16, -(-mx // CAP_GRAN) * CAP_GRAN))
    caps = tuple(caps)
    offs = np.zeros(EPC + 1, np.int64)
    offs[1:] = np.cumsum(caps)
    CTOT = int(offs[-1])
    core_of = np.empty(E, np.int64)
    pos_of = np.empty(E, np.int64)
    for c in range(N_CORES):
        for j in range(EPC):
            core_of[assign[c, j]] = c
            pos_of[assign[c, j]] = j

    # ---- dispatch: per-expert capacity-padded token blocks
    order = np.argsort(flat_e, kind="stable")
    starts = np.zeros(E + 1, np.int64)
    starts[1:] = np.cumsum(counts)
    se = flat_e[order]
    pos = np.arange(TOP_K * T) - starts[se]
    core = core_of[se]
    col = offs[pos_of[se]] + pos             # column in that core's xT
    tok = flat_t[order]

    gidx = np.zeros((N_CORES, CTOT), np.int64)
    for c in range(N_CORES):
        msel = core == c
        gidx[c, col[msel]] = tok[msel]

    np_w = _np_dt(dt_w)
    np_a = _np_dt(dt_a)
    xf_a = xf.astype(np_a, copy=False)
    # partition-major weights: w1[e, p, ht, dt, hi], w2[e, p, dt, ht, di]
    W1t = W1.reshape(E, ND, 128, NH, 128).transpose(0, 2, 3, 1, 4).astype(np_w, copy=False)
    W2t = W2.reshape(E, NH, 128, ND, 128).transpose(0, 2, 3, 1, 4).astype(np_w, copy=False)
    # pre-transpose biases to [p, e, col_tile]
    b1t = b1.reshape(E, NH, 128).transpose(2, 0, 1)
    b2t = b2.reshape(E, ND, 128).transpose(2, 0, 1)

    in_maps = []
    for c in range(N_CORES):
        es = assign[c]
        # xT[p, s] with s = ND*offs[e] + dt*cap_e + t  (per-expert blocks,
        # each [128, ND, cap] partition-major)
        xTc = np.empty((128, ND * CTOT), np_a)
        for j in range(EPC):
            blk = xf_a[gidx[c, offs[j]:offs[j + 1]]]        # [cap, D]
            blk = blk.reshape(caps[j], ND, 128).transpose(2, 1, 0)
            xTc[:, ND * offs[j]: ND * offs[j + 1]] = blk.reshape(128, -1)
        in_maps.append({
            "xT": xTc,
            "w1": np.ascontiguousarray(W1t[es]),
            "w2": np.ascontiguousarray(W2t[es]),
            "b1": np.ascontiguousarray(b1t[:, es]),
            "b2": np.ascontiguousarray(b2t[:, es]),
        })

    nc = _get_nc(caps, dt_w, dt_a, dt_y)
    trace = os.environ.get("MOE_TRACE", "0") == "1"
    res = bass_utils.run_bass_kernel_spmd(
        nc, in_maps, core_ids=list(range(N_CORES)), trace=trace,
    )
    LAST_EXEC_TIME_NS = res.exec_time_ns

    # ---- combine: gather each (token, k) contribution, weight, and sum
    # yT[p, s] -> per-expert [128, ND, cap] -> [cap, D] token-major
    Ystack = np.empty((N_CORES, CTOT, D), np.float32)
    for c in range(N_CORES):
        yc = np.asarray(res.results[c]["yT"], dtype=np.float32)
        for j in range(EPC):
            blk = yc[:, ND * offs[j]: ND * offs[j + 1]].reshape(128, ND, caps[j])
            Ystack[c, offs[j]:offs[j + 1]] = blk.transpose(2, 1, 0).reshape(caps[j], D)
    contrib = Ystack[core, col]              # [k*T, D] (sorted order)
    inv = np.empty_like(order)
    inv[order] = np.arange(TOP_K * T)
    contrib = contrib[inv].reshape(T, TOP_K, D)
    y = (contrib * w[:, :, None]).sum(1).astype(np.float32)
    return y.reshape(orig_shape)
